# revision 21
# baseline (speedup 1.0000x reference)
"""Trainium2 Bass kernel for ExactSequenceAttention (v2).

Reference math (B=4, N=2048, DIM=2048, H=1, hd=2048, S=2048):
    qkv = x @ qkv_w.T + qkv_b -> q, k, v
    attn = softmax(q @ k.T / sqrt(hd))
    ker  = (q @ sp_w.T + sp_b) @ kc_w.T + kc_b
    img  = (k @ sp_w.T + sp_b) @ ic_w.T + ic_b
    seqw = softmax((ker @ img.T / sqrt(S)) * mask)
    y    = 0.5*(attn + seqw) @ v;  out = y @ proj_w.T + proj_b

Algebraic fold (kills the img tensor entirely):
    ker @ img.T = (ker @ Wimg.T) @ k.T + outer(ker @ bimg, 1)
    with Wimg = sp_w.T@ic_w.T. Define kerW = x @ (Wq.T@Wker@Wimg.T) + bbig
    (exact), c = x @ (Wq.T@Wker@bimg) + const. Then
    seq_scores = (kerW @ k.T + outer(c, 1)) / sqrt(S)
    so BOTH score paths contract against the same k, and the whole seq
    branch costs one extra x-projection instead of ker+img+extra gather.

Sharding: 8 cores = 4 batches x 2 sequence halves. Core 2b+h owns query
rows [h*1024,(h+1)*1024) of batch b, computes k8/v for the same rows,
pair-AllGathers them (groups [0,1],[2,3],[4,5],[6,7]).

Dtypes: q/k/kerW projections and both NxN score matmuls run in fp8-e4m3
with DoubleRow perf mode (2 contraction rows/partition) — inputs are
host/device scaled into fp8 range and descaled via the exp() activation
scale. v, y=P@v and the out projection stay bf16 (fp8 there fails the
2e-2 gate; measured on CPU sim). All scores are computed transposed
(keys on partitions); softmax denominators come from a ones-row matmul;
normalization is folded into the combined weight tensor PT before a
single yT/proj chain. exp() needs no max subtraction (scores are O(1)).
"""
import math
import sys

sys.path.insert(0, "/opt/trn_rl_repo")

import numpy as np

P = 128
FD = 512        # matmul moving free dim / nb block width

DIM = 2048
B, N = 4, 2048
N_CORES = 8
GROUPS = [[0, 1], [2, 3], [4, 5], [6, 7]]

# fp8 scale plan (see module docstring):
#   x8 = fp8(x)                  (std 1.0)
#   Wq8 = fp8(32*Wq),  q8 = (psQ*(SA/32) + bq*SA)          SA=16
#   Wk8 = fp8(32*Wk),  k8 = (psK*(SK/(32*sqrt(hd))) + bk*SK/sqrt(hd)) SK=32
#   Wf8 = fp8(256*Wbig), f8 = (psF*(SF/256) + bbig*SF)     SF=16
#   psA = q8*k8' = (SA*SK/sqrt(hd)) * q.k  -> exp scale 1/(SA*SK)
#   psS likewise; c enters via DVE add of 512*c/sqrt(S).
SA, SK, SF = 16.0, 32.0, 16.0
SSC = SA * SK            # 512: score descale


def build_nc(D=DIM, NQ=N // 2, NM=N, repeat=1):
    import concourse.bacc as bacc
    import concourse.mybir as mybir
    import concourse.tile as tile
    from concourse import tile_utils
    from contextlib import ExitStack

    tile_utils.max_sbuf_usage = 204 * 1024

    F32 = mybir.dt.float32
    BF16 = mybir.dt.bfloat16
    FP8 = mybir.dt.float8e4
    AX = mybir.AluOpType
    EXP = mybir.ActivationFunctionType.Exp
    DR = mybir.MatmulPerfMode.DoubleRow

    DT = D // P          # 16 feature-dim tiles
    DB = D // FD         # 4  feature-dim blocks
    MT = NM // P         # 16 key chunks (gathered)
    NBL = NQ // FD       # 2  query blocks
    NF = FD
    NMH = NM // 2        # local (own-half) key rows
    MTH = NMH // P       # 8  local key chunks
    LCH = MT // 2        # key chunks per half

    nc = bacc.Bacc("TRN2", target_bir_lowering=False, debug=False,
                   num_devices=N_CORES)

    def din(name, shape, dt=F32):
        return nc.dram_tensor(name, list(shape), dt, kind="ExternalInput")

    x8_d = din("x8", (D, NQ), FP8)       # x[b].T own-half cols, fp8
    xz_d = din("xz", (NM, D), BF16)      # FULL x[b] row-major (Z path)
    Wq8 = din("Wq8", (DT, D, P), FP8)    # [dt][c_in][d_out]
    Wk8 = din("Wk8", (DT, D, P), FP8)
    Wf8 = din("Wf8", (DT, D, P), FP8)
    WvP = din("WvP", (DT, D, P), BF16)   # [dv][c_in][d_out] P-wide
    PwT = din("PwT", (DT, D, P), BF16)   # [ct][d_in][c_out]
    bqs_d = din("bqs", (P, DT))          # bq*SA
    bks_d = din("bks", (P, DT))          # bk*SK/sqrt(hd)
    bfs_d = din("bfs", (P, DT))          # bbig*SF
    pb_d = din("pb", (P, DT))
    bvd_d = din("bvd", (P, DT))          # bv strip-tiled
    mask_d = din("maskS", (P, MT))       # seq_mask/SSC tiled
    cB_d = din("cB", (P, NQ))            # 512*c/sqrt(S) bcast along parts
    ones16_d = din("ones16", (P, 1), BF16)

    outT = nc.dram_tensor("outT", [D, NQ], F32, kind="ExternalOutput")

    def ckload(dst, src_2d, cols, chunks=1):
        """Load a (P, DT, w) feature-major tile in `chunks` DMAs."""
        chunks = min(chunks, DT)
        gsz = DT // chunks
        for g in range(chunks):
            nc.sync.dma_start(
                dst[:, g * gsz:(g + 1) * gsz, :],
                src_2d[g * gsz * P:(g + 1) * gsz * P, cols]
                .bitcast(dst.dtype).rearrange("(o p) w -> p o w", p=P))

    with tile.TileContext(nc) as tc:
        with ExitStack() as ctx:
            consts = ctx.enter_context(tc.tile_pool(name="consts", bufs=1))
            dram = ctx.enter_context(
                tc.tile_pool(name="dram", bufs=1, space="DRAM"))

            bqs = consts.tile([P, DT], F32)
            bks = consts.tile([P, DT], F32)
            bfs = consts.tile([P, DT], F32)
            pb = consts.tile([P, DT], F32)
            maskS = consts.tile([P, MT], F32)
            bvd = consts.tile([P, DT], F32)
            cB = consts.tile([P, NQ], F32)
            ones16 = consts.tile([P, 1], BF16)
            nc.sync.dma_start(bqs[:], bqs_d[:])
            nc.sync.dma_start(bks[:], bks_d[:])
            nc.sync.dma_start(bfs[:], bfs_d[:])
            nc.sync.dma_start(pb[:], pb_d[:])
            nc.sync.dma_start(maskS[:], mask_d[:])
            nc.sync.dma_start(bvd[:], bvd_d[:])
            nc.sync.dma_start(cB[:], cB_d[:])
            nc.sync.dma_start(ones16[:], ones16_d[:])

            # k8: [chunk(8)][p(c_in)][dt][m(128)] fp8 -- chunk-contiguous
            k8_h = dram.tile([MTH, P, DT, P], FP8)
            k8_g = dram.tile([2, MTH, P, DT, P], FP8)
            # v: [mb(2)][db(4)][m_p(128)][mi(4)][d(512)] bf16 -- matches
            # the staging tile exactly so stores are contiguous DMAs
            MBH, MFB = 2, FD

            def pair_gather(half_blk, gath_blk):
                nc.gpsimd.collective_compute(
                    "AllGather", mybir.AluOpType.bypass,
                    replica_groups=GROUPS,
                    ins=[half_blk[:]], outs=[gath_blk[:]])

            def fp8_pass(x8, Wsrc, out_cb, wpool, ps1):
                """x8 @ W in fp8 DoubleRow; out_cb(dt, nb, nsl, ps)."""
                for dt in range(DT):
                    w = wpool.tile([P, DT, P], FP8, tag="w", name="w")
                    ckload(w, Wsrc[dt], slice(0, P))
                    for nb in range(NBL):
                        nsl = slice(nb * NF, (nb + 1) * NF)
                        ps = ps1.tile([P, NF], F32, tag="ps", name="ps")
                        for c2 in range(DT // 2):
                            nc.tensor.matmul(
                                ps[:], w[:, 2 * c2:2 * c2 + 2, :],
                                x8[:, 2 * c2:2 * c2 + 2, nsl],
                                start=(c2 == 0), stop=(c2 == DT // 2 - 1),
                                perf_mode=DR)
                        out_cb(dt, nb, nsl, ps)

            for _rep in range(repeat):
                with ExitStack() as rep:
                    PTpool = rep.enter_context(
                        tc.tile_pool(name="PTp", bufs=1))
                    PTs = [PTpool.tile([P, MT, NF], BF16, tag=f"PT{i}",
                                       name=f"PT{i}")
                           for i in range(NBL)]
                    xzh1 = PTpool.tile([P, LCH, DT, P], BF16, tag="xzh1")

                    with ExitStack() as front:
                        qk_pool = front.enter_context(
                            tc.tile_pool(name="qkp", bufs=1))
                        qT8 = qk_pool.tile([P, DT, NQ], FP8, tag="qT8")
                        fT8 = qk_pool.tile([P, DT, NQ], FP8, tag="fT8")

                        with ExitStack() as sx:
                            xpool = sx.enter_context(
                                tc.tile_pool(name="xp", bufs=1))
                            x8 = xpool.tile([P, DT, NQ], FP8, tag="x8")
                            for cb in range(NBL):
                                nc.sync.dma_start(
                                    x8[:, :, cb * NF:(cb + 1) * NF],
                                    x8_d[:, cb * NF:(cb + 1) * NF]
                                    .rearrange("(o p) w -> p o w", p=P))

                            # ==== Stage 1a: k8 (fp8 DoubleRow) + gather ====
                            with ExitStack() as s1:
                                wpool = s1.enter_context(
                                    tc.tile_pool(name="w1a", bufs=3))
                                ps1 = s1.enter_context(
                                    tc.tile_pool(name="ps1a", bufs=4,
                                                 space="PSUM"))
                                tmps = s1.enter_context(
                                    tc.tile_pool(name="t1a", bufs=4))

                                def k_out(dt, nb, nsl, ps):
                                    t = tmps.tile([P, NF], FP8, tag="t",
                                                  name="t")
                                    nc.any.tensor_scalar(
                                        out=t[:], in0=ps[:],
                                        scalar1=SK / (32.0 * math.sqrt(D)),
                                        scalar2=bks[:, dt:dt + 1],
                                        op0=AX.mult, op1=AX.add)
                                    nc.sync.dma_start(
                                        k8_h[nb * 4:(nb + 1) * 4, :, dt, :]
                                        .rearrange("mi p m -> p mi m"),
                                        t[:].rearrange(
                                            "p (mi m) -> p mi m", mi=4))

                                fp8_pass(x8, Wk8, k_out, wpool, ps1)
                                pair_gather(k8_h, k8_g)

                            # ==== xz loads (first half) ====
                            for mt in range(LCH):
                                nc.sync.dma_start(
                                    xzh1[:, mt, :, :],
                                    xz_d[mt * P:(mt + 1) * P, :]
                                    .rearrange("p (o c) -> p o c", o=DT))

                            # ==== Stage 1c+1d: qT8, fT8 (SBUF-resident) ====
                            with ExitStack() as s1:
                                wpool = s1.enter_context(
                                    tc.tile_pool(name="w1c", bufs=3))
                                ps1 = s1.enter_context(
                                    tc.tile_pool(name="ps1c", bufs=4,
                                                 space="PSUM"))
                                for Wsrc, dst, dsc, bias in (
                                        (Wq8, qT8, SA / 32.0, bqs),
                                        (Wf8, fT8, SF / 256.0, bfs)):
                                    def qf_out(dt, nb, nsl, ps,
                                               dst=dst, dsc=dsc, bias=bias):
                                        nc.any.tensor_scalar(
                                            out=dst[:, dt, nsl], in0=ps[:],
                                            scalar1=dsc,
                                            scalar2=bias[:, dt:dt + 1],
                                            op0=AX.mult, op1=AX.add)
                                    fp8_pass(x8, Wsrc, qf_out, wpool, ps1)

                        # ==== Stage 2a: scores/softmax -> PT[nb] ====
                        with ExitStack() as s2:
                            blk = s2.enter_context(
                                tc.tile_pool(name="blk", bufs=1))
                            kres = s2.enter_context(
                                tc.tile_pool(name="kres", bufs=1))
                            small = s2.enter_context(
                                tc.tile_pool(name="small", bufs=4))
                            psAS = s2.enter_context(
                                tc.tile_pool(name="psAS", bufs=6,
                                             space="PSUM"))
                            psSums = s2.enter_context(
                                tc.tile_pool(name="psSums", bufs=1,
                                             space="PSUM"))

                            k8r = kres.tile([P, MT, DT, P], FP8, tag="k8r")

                            for nb in range(NBL):
                                nsl = slice(nb * NF, (nb + 1) * NF)
                                expA = blk.tile([P, MT, NF], BF16, tag="expA",
                                                name="expA")
                                expS = blk.tile([P, MT, NF], BF16, tag="expS",
                                                name="expS")
                                sumA = psSums.tile([1, NF], F32, tag="sumA",
                                                   name="sumA")
                                sumS = psSums.tile([1, NF], F32, tag="sumS",
                                                   name="sumS")

                                # A path (+ k8 chunk loads on first block)
                                for mt in range(MT):
                                    if nb == 0:
                                        h, l = divmod(mt, LCH)
                                        nc.sync.dma_start(
                                            k8r[:, mt, :, :], k8_g[h][l])
                                    psA = psAS.tile([P, NF], F32, tag="psA",
                                                    name="psA")
                                    for c2 in range(DT // 2):
                                        nc.tensor.matmul(
                                            psA[:],
                                            k8r[:, mt, 2 * c2:2 * c2 + 2, :],
                                            qT8[:, 2 * c2:2 * c2 + 2, nsl],
                                            start=(c2 == 0),
                                            stop=(c2 == DT // 2 - 1),
                                            perf_mode=DR)
                                    nc.scalar.activation(
                                        expA[:, mt, :], psA[:], EXP,
                                        scale=1.0 / SSC)
                                    if mt > 0:
                                        nc.tensor.matmul(
                                            sumA[:], ones16[:],
                                            expA[:, mt - 1, :],
                                            start=(mt == 1), stop=False,
                                            skip_group_check=True)
                                nc.tensor.matmul(
                                    sumA[:], ones16[:], expA[:, MT - 1, :],
                                    start=False, stop=True,
                                    skip_group_check=True)

                                # A normalization overlaps the S loop below
                                rcpA = small.tile([1, NF], F32, tag="rcp",
                                                  name="rcpA")
                                nc.vector.reciprocal(rcpA[:], sumA[:])
                                nc.any.tensor_scalar_mul(rcpA[:], rcpA[:], 0.5)
                                RA = small.tile([P, NF], F32, tag="RB",
                                                name="RA")
                                nc.gpsimd.partition_broadcast(RA[:], rcpA[:])
                                PT = PTs[nb]
                                for mt in range(MT):
                                    nc.any.tensor_tensor(
                                        PT[:, mt, :], expA[:, mt, :], RA[:],
                                        AX.mult)

                                # S path
                                for mt in range(MT):
                                    psS = psAS.tile([P, NF], F32, tag="psA",
                                                    name="psS")
                                    for c2 in range(DT // 2):
                                        nc.tensor.matmul(
                                            psS[:],
                                            k8r[:, mt, 2 * c2:2 * c2 + 2, :],
                                            fT8[:, 2 * c2:2 * c2 + 2, nsl],
                                            start=(c2 == 0),
                                            stop=(c2 == DT // 2 - 1),
                                            perf_mode=DR)
                                    nc.any.tensor_tensor(
                                        psS[:], psS[:], cB[:, nsl], AX.add)
                                    nc.scalar.activation(
                                        expS[:, mt, :], psS[:], EXP,
                                        scale=maskS[:, mt:mt + 1])
                                    if mt > 0:
                                        nc.tensor.matmul(
                                            sumS[:], ones16[:],
                                            expS[:, mt - 1, :],
                                            start=(mt == 1), stop=False,
                                            skip_group_check=True)
                                nc.tensor.matmul(
                                    sumS[:], ones16[:], expS[:, MT - 1, :],
                                    start=False, stop=True,
                                    skip_group_check=True)

                                rcpS = small.tile([1, NF], F32, tag="rcp",
                                                  name="rcpS")
                                nc.vector.reciprocal(rcpS[:], sumS[:])
                                nc.any.tensor_scalar_mul(rcpS[:], rcpS[:], 0.5)
                                RS = small.tile([P, NF], F32, tag="RB",
                                                name="RS")
                                nc.gpsimd.partition_broadcast(RS[:], rcpS[:])
                                for mt in range(MT):
                                    nc.any.tensor_tensor(
                                        expS[:, mt, :], expS[:, mt, :], RS[:],
                                        AX.mult)
                                    nc.any.tensor_tensor(
                                        PT[:, mt, :], PT[:, mt, :],
                                        expS[:, mt, :], AX.add)

                    # ==== Stage 2b: Z = x_full^T @ PT; yT = WvP.T @ Z ====
                    with ExitStack() as s3:
                        ypool = s3.enter_context(
                            tc.tile_pool(name="yp", bufs=1))
                        stream = s3.enter_context(
                            tc.tile_pool(name="stm2", bufs=3))
                        psY = s3.enter_context(
                            tc.tile_pool(name="psY", bufs=4, space="PSUM"))
                        tmp2 = s3.enter_context(
                            tc.tile_pool(name="t2", bufs=4))
                        xzh2 = ypool.tile([P, LCH, DT, P], BF16, tag="xzh2")
                        for mt in range(LCH):
                            nc.sync.dma_start(
                                xzh2[:, mt, :, :],
                                xz_d[(LCH + mt) * P:(LCH + mt + 1) * P, :]
                                .rearrange("p (o c) -> p o c", o=DT))
                        Zt = ypool.tile([P, DT, NQ], BF16, tag="Zt")
                        yTs = [ypool.tile([P, DT, NF], BF16, tag=f"yT{i}",
                                          name=f"yT{i}")
                               for i in range(NBL)]
                        # Z[c, n] = sum_m x[m, c] * PT[m, n]
                        for dt in range(DT):
                            for nb in range(NBL):
                                nsl = slice(nb * NF, (nb + 1) * NF)
                                ps = psY.tile([P, NF], F32, tag="ps",
                                              name="psz")
                                for mt in range(MT):
                                    xsrc = xzh1 if mt < LCH else xzh2
                                    nc.tensor.matmul(
                                        ps[:], xsrc[:, mt % LCH, dt, :],
                                        PTs[nb][:, mt, :],
                                        start=(mt == 0),
                                        stop=(mt == MT - 1))
                                nc.any.tensor_copy(
                                    out=Zt[:, dt, nsl], in_=ps[:])
                        # yT[d, n] = sum_c Wv[d, c] * Z[c, n] + bv[d]
                        for dv in range(DT):
                            wv = stream.tile([P, DT, P], BF16, tag="stm",
                                             name="wv")
                            ckload(wv, WvP[dv], slice(0, P))
                            for nb in range(NBL):
                                nsl = slice(nb * NF, (nb + 1) * NF)
                                ps = psY.tile([P, NF], F32, tag="ps",
                                              name="psy")
                                for ck in range(DT):
                                    nc.tensor.matmul(
                                        ps[:], wv[:, ck, :],
                                        Zt[:, ck, nsl],
                                        start=(ck == 0),
                                        stop=(ck == DT - 1))
                                nc.any.tensor_scalar(
                                    out=yTs[nb][:, dv, :], in0=ps[:],
                                    scalar1=bvd[:, dv:dv + 1],
                                    scalar2=None, op0=AX.add)

                        # ==== Stage 2c: outT = PwT.T @ yT + pb ====
                        for ct in range(DT):
                            pw = stream.tile([P, DT, P], BF16, tag="stm",
                                             name="pw")
                            ckload(pw, PwT[ct], slice(0, P))
                            for nb in range(NBL):
                                nsl = slice(nb * NF, (nb + 1) * NF)
                                ps = psY.tile([P, NF], F32, tag="ps",
                                              name="pso")
                                for dt in range(DT):
                                    nc.tensor.matmul(
                                        ps[:], pw[:, dt, :],
                                        yTs[nb][:, dt, :],
                                        start=(dt == 0),
                                        stop=(dt == DT - 1))
                                t = tmp2.tile([P, NF], F32, tag="t", name="t")
                                nc.any.tensor_scalar(
                                    out=t[:], in0=ps[:],
                                    scalar1=pb[:, ct:ct + 1],
                                    scalar2=None, op0=AX.add)
                                nc.sync.dma_start(
                                    outT[ct * P:(ct + 1) * P, nsl], t[:])

    nc.compile()
    return nc


def prep_inputs(x, qkv_w, qkv_b, proj_w, proj_b, sp_w, sp_b, kc_w, kc_b,
                ic_w, ic_b, seq_mask, D=DIM, NQ=N // 2, NM=N):
    """Host-side weight folding + per-core input maps."""
    import ml_dtypes
    F8 = ml_dtypes.float8_e4m3
    BF = ml_dtypes.bfloat16
    DT = D // P
    MT = NM // P
    f32 = np.float32
    f64 = np.float64

    S = D
    rs_S = 1.0 / math.sqrt(S)

    Wq = qkv_w[0:D].astype(f64)
    Wk = qkv_w[D:2 * D].astype(f64)
    Wv = qkv_w[2 * D:3 * D].astype(f64)
    bq = qkv_b[0:D].astype(f64)
    bk = qkv_b[D:2 * D].astype(f64)
    bv = qkv_b[2 * D:3 * D].astype(f64)

    def strip_tile(WT, width, dt):
        # (D, D) [c_in, d_out] -> (D//width, D, width) [tile][c_in][d_out]
        return np.ascontiguousarray(
            WT.reshape(D, D // width, width).transpose(1, 0, 2)).astype(dt)

    # seq-path folds
    Wker = sp_w.T.astype(f64) @ kc_w.T.astype(f64)
    bker = sp_b.astype(f64) @ kc_w.T.astype(f64) + kc_b.astype(f64)
    Wimg = sp_w.T.astype(f64) @ ic_w.T.astype(f64)
    bimg = sp_b.astype(f64) @ ic_w.T.astype(f64) + ic_b.astype(f64)
    Wfold = Wker @ Wimg.T                  # (D, D)
    bfold = bker @ Wimg.T                  # (D,)
    u = Wker @ bimg                        # (D,)
    cconst = float(bker @ bimg)

    WbigT = Wq.T @ Wfold                   # kerW = x @ WbigT + bbig
    bbig = bq @ Wfold + bfold
    uq = Wq.T @ u                          # c = x @ uq + (bq@u + cconst)
    cc0 = float(bq @ u) + cconst

    Wq8 = strip_tile(Wq.T * 32.0, P, F8)
    Wk8 = strip_tile(Wk.T * 32.0, P, F8)
    Wf8 = strip_tile(WbigT * 256.0, P, F8)
    WvP = strip_tile(Wv.T, P, BF)
    PwT = strip_tile(proj_w.T.astype(f64), P, BF)

    bqs = np.ascontiguousarray((bq * SA).reshape(DT, P).T).astype(f32)
    bks = np.ascontiguousarray(
        (bk * (SK / math.sqrt(D))).reshape(DT, P).T).astype(f32)
    bfs = np.ascontiguousarray((bbig * SF).reshape(DT, P).T).astype(f32)
    pb_h = np.ascontiguousarray(
        proj_b.astype(f64).reshape(DT, P).T).astype(f32)
    bvd = np.ascontiguousarray(bv.reshape(DT, P).T).astype(f32)
    maskS = np.ascontiguousarray(
        np.asarray(seq_mask, dtype=f64)[0].reshape(MT, P).T / SSC).astype(f32)
    ones16_h = np.ones((P, 1), dtype=BF)

    shared = dict(Wq8=Wq8, Wk8=Wk8, Wf8=Wf8, WvP=WvP, PwT=PwT,
                  bqs=bqs, bks=bks, bfs=bfs, pb=pb_h, bvd=bvd,
                  maskS=maskS, ones16=ones16_h)

    in_maps = []
    for core in range(N_CORES):
        b, h = divmod(core, 2)
        xb = np.asarray(x[b], dtype=f64)
        xT = np.ascontiguousarray(xb.T[:, h * NQ:(h + 1) * NQ])
        c = (xT.T @ uq + cc0) * (SSC * rs_S)      # [NQ]
        m = dict(shared)
        m["x8"] = xT.astype(F8)
        m["xz"] = np.ascontiguousarray(xb).astype(BF)
        m["cB"] = np.ascontiguousarray(
            np.broadcast_to(c.astype(f32), (P, NQ)))
        in_maps.append(m)
    return in_maps


_NC_CACHE = {}


def kernel(**inputs):
    from concourse.bass_utils import run_bass_kernel_spmd

    key = "full"
    if key not in _NC_CACHE:
        _NC_CACHE[key] = build_nc()
    nc = _NC_CACHE[key]

    NQ = N // 2
    in_maps = prep_inputs(**inputs)
    res = run_bass_kernel_spmd(nc, in_maps, core_ids=list(range(N_CORES)))
    out = np.empty((B, N, DIM), dtype=np.float32)
    for core in range(N_CORES):
        b, h = divmod(core, 2)
        out[b, h * NQ:(h + 1) * NQ, :] = res.results[core]["outT"].T
    return out
